# revision 1
# baseline (speedup 1.0000x reference)
"""L1 pairwise distance kernel for Trainium2, 8 NeuronCores.

res[i, j] = sum_d |x1c[i, d] - x2c[j, d]|,  x1c/x2c centered by mean(x1).

Strategy (per core, i-slab of 256 rows of x1):
  - Layout: SBUF partitions = (copy in {0,1}) * 64 + d.  `x2s` holds x2c^T
    stacked twice: x2s[64*c + d, j] = x2c[j, d]  -> [128, 2048] fp32,
    replicated on every core (data-parallel over x1 rows, per the
    sharding hint).
  - For each pair g of x1 rows (i = i0 + 2g, i0 + 2g + 1) one engine op
    produces the whole absdiff tile A[128, 2048]:
        ScalarE:  A = Abs(x2s + bias[:, g])          (bias = -x1 pair, per-partition)
        VectorE:  A = (x2s + bias[:, g]) ; A &= 0x7fffffff  (2 ops)
    The two engines split the pairs so they run concurrently.
  - TensorE reduces over d with a shifted one-hot mask matmul:
        psum[m, j] += sum_k mask_g[k, m] * A[k, j]
    mask_g[k, m] = 1 iff m == 2g + k//64, so psum row m accumulates
    exactly row i0+128h+m of the output.  Masks are 128-column slices of
    one [128, 254] base tensor; rhs is the absdiff tile bitcast to
    float32r (full-rate PE, ~fp32 precision).  64 pairs accumulate into
    one [128, 2048] PSUM tile (4 banks), two halves use the 8 banks.
  - DVE copies PSUM -> SBUF, DMA writes 128 contiguous output rows.

Self-contained: hardcodes shapes from the problem spec.
"""

import numpy as np

import bass_rust
import concourse.bass as bass
import concourse.tile as tile
from concourse import mybir
import concourse.bass_utils as bu

N1 = 2048
N2 = 2048
D = 64
NCORES = 8
IPC = N1 // NCORES          # 256 x1 rows per core
NPAIR_HALF = 64             # pairs per 128-partition PSUM tile
JCH = 512                   # matmul free-dim chunk (one PSUM bank)
F32 = mybir.dt.float32
F32R = mybir.dt.float32r
A = mybir.AluOpType

F16 = mybir.dt.float16
I16 = mybir.dt.int16

# Pairs handled by the Vector engine (rest on ScalarE): g % 16 in DVE_SLOTS.
# DVE pairs read an fp16 copy of x2s so tensor_scalar runs in 4x mode
# (~594ns vs 1127ns per op), then int16 AND abs, fp16-mask matmul; ACT
# pairs stay fp32->f32r.  10/16 on DVE balances the engines per the
# cost-model sweep (130.3us vs 136.5us at 8/16 fp32-source).
DVE_SLOTS = (0, 1, 3, 4, 6, 9, 10, 12, 13, 15)

_nop_counter = [0]


def _split_multi_waits(nc):
    """This container's walrus build allows one sync-wait per instruction.
    Move extra waits onto same-engine NoOps placed just before."""
    for fn in nc.m.functions:
        for blk in fn.blocks:
            out = []
            changed = False
            for inst in blk.instructions:
                si = inst.sync_info
                if si is not None and len(si.on_wait) > 1:
                    waits = list(si.on_wait)
                    for w in waits[:-1]:
                        _nop_counter[0] += 1
                        nop = mybir.InstNoOp(
                            name=f"I-waitsplit-{_nop_counter[0]}", ins=[], outs=[]
                        )
                        nop.engine = inst.engine
                        nop.sync_info = bass_rust.SyncInfo(on_wait=[w], on_update=[])
                        if inst.debug is not None:
                            nop.debug = inst.debug
                        out.append(nop)
                        nc.register_instruction(nop, overwrite=True)
                    si.on_wait = waits[-1:]
                    changed = True
                out.append(inst)
            if changed:
                blk.instructions = out


def _build():
    nc = bass.Bass()
    x2s_d = nc.dram_tensor("x2s", [128, N2], F32, kind="ExternalInput")
    x2s16_d = nc.dram_tensor("x2s16", [128, N2], F16, kind="ExternalInput")
    bias_d = nc.dram_tensor("bias", [128, IPC // 2], F32, kind="ExternalInput")
    mask_d = nc.dram_tensor("maskb", [128, 254], F32R, kind="ExternalInput")
    maskb16_d = nc.dram_tensor("maskb16", [128, 254], F16, kind="ExternalInput")
    out_d = nc.dram_tensor("out", [IPC, N2], F32, kind="ExternalOutput")

    with tile.TileContext(nc) as tc:
        with (
            tc.tile_pool(name="singles", bufs=1) as singles,
            tc.tile_pool(name="ad", bufs=6) as adpool,
            tc.tile_pool(name="ps", bufs=2, space="PSUM") as pspool,
            tc.tile_pool(name="ob", bufs=2) as outpool,
        ):
            x2s = singles.tile([128, N2], F32)
            nc.sync.dma_start(x2s[:], x2s_d[:])
            x2s16 = singles.tile([128, N2], F16)
            nc.sync.dma_start(x2s16[:], x2s16_d[:])
            bias = singles.tile([128, IPC // 2], F32)
            nc.sync.dma_start(bias[:], bias_d[:])
            maskb = singles.tile([128, 254], F32R)
            nc.sync.dma_start(maskb[:], mask_d[:])
            maskb16 = singles.tile([128, 254], F16)
            nc.sync.dma_start(maskb16[:], maskb16_d[:])
            andmask = singles.tile([128, 1], I16)
            nc.vector.memset(andmask[:], 0x7FFF)

            for h in range(2):
                ps = pspool.tile([128, N2], F32)
                for g in range(NPAIR_HALF):
                    col = h * NPAIR_HALF + g
                    if g % 16 in DVE_SLOTS:
                        adb = adpool.tile([128, N2], F16, tag="adb")
                        nc.vector.tensor_scalar(
                            out=adb[:], in0=x2s16[:],
                            scalar1=bias[:, col : col + 1], scalar2=None,
                            op0=A.add,
                        )
                        adbi = adb[:].bitcast(I16)
                        nc.vector.tensor_scalar(
                            out=adbi, in0=adbi,
                            scalar1=andmask[:], scalar2=None,
                            op0=A.bitwise_and,
                        )
                        mg = maskb16[:, 126 - 2 * g : 254 - 2 * g]
                        rhs = adb
                    else:
                        ad = adpool.tile([128, N2], F32R, tag="ad")
                        nc.scalar.activation(
                            out=ad[:], in_=x2s[:],
                            func=mybir.ActivationFunctionType.Abs,
                            bias=bias[:, col : col + 1], scale=1.0,
                        )
                        mg = maskb[:, 126 - 2 * g : 254 - 2 * g]
                        rhs = ad
                    for jc in range(N2 // JCH):
                        nc.tensor.matmul(
                            ps[:, jc * JCH : (jc + 1) * JCH],
                            mg,
                            rhs[:, jc * JCH : (jc + 1) * JCH],
                            start=(g == 0),
                            stop=(g == NPAIR_HALF - 1),
                        )
                ob = outpool.tile([128, N2], F32)
                nc.vector.tensor_copy(ob[:], ps[:])
                nc.sync.dma_start(out_d[h * 128 : (h + 1) * 128, :], ob[:])
    _split_multi_waits(nc)
    return nc


_cached_nc = None


def _prep_inputs(x1, x2):
    x1 = np.asarray(x1, dtype=np.float32)
    x2 = np.asarray(x2, dtype=np.float32)
    adj = x1.mean(axis=0, dtype=np.float32).astype(np.float32)
    x1c = x1 - adj
    x2c = x2 - adj

    x2s = np.tile(np.ascontiguousarray(x2c.T), (2, 1)).astype(np.float32)  # [128, N2]

    maskb = np.zeros((128, 254), dtype=np.float32)
    k = np.arange(128)
    maskb[k, 126 + k // 64] = 1.0
    maskb16 = maskb.astype(np.float16)

    in_maps = []
    for c in range(NCORES):
        sl = x1c[c * IPC : (c + 1) * IPC]          # [256, 64]
        b = -np.transpose(sl.reshape(IPC // 2, 2, D), (1, 2, 0)).reshape(128, IPC // 2)
        in_maps.append({
            "x2s": x2s,
            "x2s16": x2s.astype(np.float16),
            "bias": np.ascontiguousarray(b, dtype=np.float32),
            "maskb": maskb,
            "maskb16": maskb16,
        })
    return in_maps


def run(x1, x2, trace=False):
    global _cached_nc
    if _cached_nc is None:
        _cached_nc = _build()
    in_maps = _prep_inputs(x1, x2)
    r = bu.run_bass_kernel_spmd(
        _cached_nc, in_maps, core_ids=list(range(NCORES)), trace=trace
    )
    out = np.concatenate([r.results[c]["out"] for c in range(NCORES)], axis=0)
    return out, r


def kernel(x1, x2):
    out, _ = run(x1, x2, trace=False)
    return out



# revision 10
# speedup vs baseline: 1.1104x; 1.1104x over previous
"""L1 pairwise distance kernel for Trainium2, 8 NeuronCores.

res[i, j] = sum_d |x1c[i, d] - x2c[j, d]|,  x1c/x2c centered by mean(x1).

Strategy (per core, i-slab of 256 rows of x1, data-parallel over x1 rows):
  Two 128-row halves; each half's 64 row-pairs split across two paths:
  - A-path (nA pairs, DVE): fp16 absdiff tile [128, 2048] in 2 DVE ops
    (tensor_scalar add bias at 4x; int16 AND 0x7fff at 4x), reduced over
    d by fp16 one-hot mask matmuls into PSUM_A (1 col/cycle).
  - J-path (64-nA pairs, ACT): ScalarE activation Abs emits the absdiff
    tile directly in fp8e4 (1 op per pair, dtype-free cost). Pairs of
    tiles feed DoubleRow fp8 matmuls (256-deep contraction, 0.5
    cycles/row -> 4x fewer PE cycles than the fp16 path) into PSUM_J.
    fp8 quantization of |diff| bounds rel err ~1.4e-2 < 2e-2.
  - GpSimd copies PSUM->SBUF (full-tile copy then overwrite rows
    [2nA:128) from PSUM_J), one DMA per half writes 128 output rows.

Self-contained: hardcodes shapes from the problem spec.
"""

import numpy as np
import ml_dtypes

import bass_rust
import concourse.bass as bass
import concourse.tile as tile
from concourse import mybir
import concourse.bass_utils as bu

N1 = 2048
N2 = 2048
D = 64
NCORES = 8
IPC = N1 // NCORES          # 256 x1 rows per core
NPAIR_HALF = 64             # row-pairs per 128-partition half
JCH = 512                   # matmul free-dim chunk (one PSUM bank)
NA = 40                     # A-path (fp16/DVE) pairs per half
NJ = NPAIR_HALF - NA        # J-path (fp8/ACT) pairs per half (even)
NG = NJ // 2                # DoubleRow groups per half
F32 = mybir.dt.float32
F16 = mybir.dt.float16
F8 = mybir.dt.float8e4
I16 = mybir.dt.int16
A = mybir.AluOpType

_nop_counter = [0]


def _split_multi_waits(nc):
    """This container's walrus build allows one sync-wait per instruction.
    Move extra waits onto same-engine NoOps placed just before."""
    for fn in nc.m.functions:
        for blk in fn.blocks:
            out = []
            changed = False
            for inst in blk.instructions:
                si = inst.sync_info
                if si is not None and len(si.on_wait) > 1:
                    waits = list(si.on_wait)
                    for w in waits[:-1]:
                        _nop_counter[0] += 1
                        nop = mybir.InstNoOp(
                            name=f"I-waitsplit-{_nop_counter[0]}", ins=[], outs=[]
                        )
                        nop.engine = inst.engine
                        nop.sync_info = bass_rust.SyncInfo(on_wait=[w], on_update=[])
                        if inst.debug is not None:
                            nop.debug = inst.debug
                        out.append(nop)
                        nc.register_instruction(nop, overwrite=True)
                    si.on_wait = waits[-1:]
                    changed = True
                out.append(inst)
            if changed:
                blk.instructions = out


def _build():
    nc = bass.Bass()
    x2s16_d = nc.dram_tensor("x2s16", [128, N2], F16, kind="ExternalInput")
    bias_d = nc.dram_tensor("bias", [128, IPC // 2], F32, kind="ExternalInput")
    maskb16_d = nc.dram_tensor("maskb16", [128, 254], F16, kind="ExternalInput")
    base8_d = nc.dram_tensor("base8", [128, NG, 2, 128], F8, kind="ExternalInput")
    out_d = nc.dram_tensor("out", [IPC, N2], F32, kind="ExternalOutput")

    with tile.TileContext(nc) as tc:
        with (
            tc.tile_pool(name="singles", bufs=1) as singles,
            tc.tile_pool(name="ad", bufs=8) as adpool,
            tc.tile_pool(name="jd", bufs=5) as jdpool,
            tc.tile_pool(name="psa", bufs=1, space="PSUM") as psapool,
            tc.tile_pool(name="psj", bufs=1, space="PSUM") as psjpool,
            tc.tile_pool(name="ob", bufs=4) as outpool,
        ):
            x2s16 = singles.tile([128, N2], F16)
            nc.sync.dma_start(x2s16[:], x2s16_d[:])
            bias = singles.tile([128, IPC // 2], F32)
            nc.sync.dma_start(bias[:], bias_d[:])
            maskb16 = singles.tile([128, 254], F16)
            nc.sync.dma_start(maskb16[:], maskb16_d[:])
            base8 = singles.tile([128, NG, 2, 128], F8)
            nc.sync.dma_start(base8[:], base8_d[:])
            andmask = singles.tile([128, 1], I16)
            nc.vector.memset(andmask[:], 0x7FFF)

            for h in range(2):
                psA = psapool.tile([128, N2], F32)
                psJ = psjpool.tile([128, N2], F32)
                # Interleave emission so PE queue alternates A / J work.
                na_done = 0
                for g in range(NG):
                    # one DoubleRow group: 2 J-pairs
                    jt = jdpool.tile([128, 2, N2], F8, tag="jt")
                    for i in range(2):
                        pi = h * NPAIR_HALF + NA + 2 * g + i  # pair index in core
                        nc.scalar.activation(
                            out=jt[:, i, :], in_=x2s16[:],
                            func=mybir.ActivationFunctionType.Abs,
                            bias=bias[:, pi : pi + 1], scale=1.0,
                        )
                    for jc in range(N2 // JCH):
                        nc.tensor.matmul(
                            psJ[:, jc * JCH : (jc + 1) * JCH],
                            base8[:, g, :, :],
                            jt[:, :, jc * JCH : (jc + 1) * JCH],
                            start=(g == 0),
                            stop=(g == NG - 1),
                            perf_mode=mybir.MatmulPerfMode.DoubleRow,
                        )
                    # ~3-4 A-pairs per J-group keeps both pipes fed
                    na_target = min(NA, ((g + 1) * NA) // NG)
                    for a in range(na_done, na_target):
                        pi = h * NPAIR_HALF + a
                        ad = adpool.tile([128, N2], F16, tag="ad")
                        nc.vector.tensor_scalar(
                            out=ad[:], in0=x2s16[:],
                            scalar1=bias[:, pi : pi + 1], scalar2=None,
                            op0=A.add,
                        )
                        adi = ad[:].bitcast(I16)
                        nc.vector.tensor_scalar(
                            out=adi, in0=adi,
                            scalar1=andmask[:], scalar2=None,
                            op0=A.bitwise_and,
                        )
                        mg = maskb16[:, 126 - 2 * a : 254 - 2 * a]
                        for jc in range(N2 // JCH):
                            nc.tensor.matmul(
                                psA[:, jc * JCH : (jc + 1) * JCH],
                                mg,
                                ad[:, jc * JCH : (jc + 1) * JCH],
                                start=(a == 0),
                                stop=(a == NA - 1),
                            )
                        na_done = na_target

                obA = outpool.tile([128, N2], F32, tag="obA")
                obJ = outpool.tile([128, N2], F32, tag="obJ")
                nc.scalar.copy(obA[:], psA[:])
                nc.scalar.copy(obJ[:], psJ[:])
                nc.sync.dma_start(
                    out_d[h * 128 : h * 128 + 2 * NA, :], obA[0 : 2 * NA, :]
                )
                nc.sync.dma_start(
                    out_d[h * 128 + 2 * NA : (h + 1) * 128, :], obJ[2 * NA : 128, :]
                )
    _split_multi_waits(nc)
    return nc


_cached_nc = None


def _prep_inputs(x1, x2):
    x1 = np.asarray(x1, dtype=np.float32)
    x2 = np.asarray(x2, dtype=np.float32)
    adj = x1.mean(axis=0, dtype=np.float32).astype(np.float32)
    x1c = x1 - adj
    x2c = x2 - adj

    x2s = np.tile(np.ascontiguousarray(x2c.T), (2, 1)).astype(np.float32)  # [128, N2]
    x2s16 = x2s.astype(np.float16)

    # fp16 one-hot masks for the A-path (target partitions 2a, 2a+1)
    maskb = np.zeros((128, 254), dtype=np.float32)
    k = np.arange(128)
    maskb[k, 126 + k // 64] = 1.0
    maskb16 = maskb.astype(np.float16)

    # fp8 DoubleRow masks, one [128, 2, 128] block per group g:
    # base8[k, g, i, m] = 1 iff m == 2*NA + 4g + 2i + k//64
    base8 = np.zeros((128, NG, 2, 128), dtype=ml_dtypes.float8_e4m3fn)
    for g in range(NG):
        for i in range(2):
            base8[k, g, i, 2 * NA + 4 * g + 2 * i + k // 64] = 1.0

    in_maps = []
    for c in range(NCORES):
        sl = x1c[c * IPC : (c + 1) * IPC]          # [256, 64]
        b = -np.transpose(sl.reshape(IPC // 2, 2, D), (1, 2, 0)).reshape(128, IPC // 2)
        in_maps.append({
            "x2s16": x2s16,
            "bias": np.ascontiguousarray(b, dtype=np.float32),
            "maskb16": maskb16,
            "base8": base8.view(np.uint8),
        })
    return in_maps


def run(x1, x2, trace=False):
    global _cached_nc
    if _cached_nc is None:
        _cached_nc = _build()
    in_maps = _prep_inputs(x1, x2)
    r = bu.run_bass_kernel_spmd(
        _cached_nc, in_maps, core_ids=list(range(NCORES)), trace=trace
    )
    out = np.concatenate([r.results[c]["out"] for c in range(NCORES)], axis=0)
    return out, r


def kernel(x1, x2):
    out, _ = run(x1, x2, trace=False)
    return out
